# revision 10
# baseline (speedup 1.0000x reference)
"""HarmonicCausalSelfAttention on 8 TRN2 NeuronCores.

Sharding: core c -> (batch b = c//2, head-group g = c%2); each core computes
attention for 8 heads of one batch and a full-width partial of the output
projection; the host sums the two partials per batch (the rank-128 c-proj
intermediate is linear, so out = (r_g0 + r_g1) @ cB^T = part_g0 + part_g1).

Layout: everything transposed so no on-device transposes are needed.
  stage1:  t^T[rank, T] = A @ x^T       (x^T prepared host-side, bf16)
  stage2:  QT2/KT2[128 = 2 heads x 64d, hp, T] pair-stacked; V keys-major
           [keys, 128] where cols 64:128 are ALL ONES (so PV psum rows
           64:128 hold the softmax denominator replicated 64x).
  attn per head pair, per q-half (hc), per 128-key strip kb:
           S^T[keys, q] = K_kb @ Q^T   (half-array K=64; the e=0 and e=1
           head lanes use partition rows 0:64 / 64:128, emitted
           back-to-back so the PE runs them concurrently as row-group
           tiles)
           softmax exp is SPLIT ACROSS ENGINES: the e=0 lane runs native
           exp on ScalarE (PSUM->SBUF bf16); the e=1 lane runs a CUSTOM
           DVE op (EXP2_BF16_ANT, registered at import) that constructs
           the bf16 BIT PATTERN of 2^y arithmetically -- magic-constant
           floor/frac split + quadratic mantissa correction, written
           through an f32->int16 convert and bitcast back to bf16
           (max rel err 5.4e-3, rms 2.2e-3; scores are pre-scaled by
           kappa = 16/ln2 host-side via the Q projection so psum already
           holds y*128 = log2(e^{S/8})*128). Causal diag masking is
           POST-exp by a bf16 0/1 multiply on GpSimd.
           PV: psum[128, 512q] += V_kb^T @ P^T_kb; normalization: ScalarE
           stages the replicated denominator rows 64:128 -> SBUF, then one
           custom DVE op (SCALE_RECIP_ANT: bitwise-NOT reciprocal seed +
           1 Newton step + multiply, 6 stages, max rel err 1.7e-3)
           computes Y^T = ytilde * (1/den) straight out of PSUM.
  c_proj:  r^T[rank, T] = sum_hp cA_hp2[128d x rank] @ YT2_hp; out chunks
           = r^T.T @ cB^T, cast bf16 (ScalarE/DVE alternating), DMA out.
PSUM-evacuation copies everywhere are alternated between ScalarE and
VectorE so neither engine paces the loop; TensorE is the critical path.
Output returned in bf16; host sums partials in f32.

Measured on 8-core trn2 (neuron-profile): 608 us (session-start)
-> 338 us (prev session) -> this version targets ~150 us.
"""

import math

import numpy as np
import ml_dtypes

import concourse.bass as bass
from concourse import bacc
import concourse.mybir as mybir
from concourse.tile import TileContext
from concourse.bass_utils import run_bass_kernel_spmd

B, T, C = 4, 2048, 1024
NH, HD = 16, 64
RANK = 128
NCORES = 8
HPC = 8          # heads per core
NPAIR = 4        # head pairs per core
G = 512          # C columns per head group
P = 128
F32 = mybir.dt.float32
BF16 = mybir.dt.bfloat16
I16 = mybir.dt.int16
BF = ml_dtypes.bfloat16

# scores arrive in psum pre-scaled so that psum = y*128, y = log2(exp(S/8))
KAPPA = 16.0 / math.log(2.0)          # folded into qB host-side
LN2_OVER_128 = math.log(2.0) / 128.0  # ScalarE exp scale for the same psum

_NC_CACHE = None

# ---------------------------------------------------------------------------
# custom DVE ops, registered into concourse.dve_ops at import time
# ---------------------------------------------------------------------------

# EXP2_BF16_ANT constants (fit numerically; see docstring)
EXP2_A = 0.002689572895776347
EXP2_C3 = -19931.876633283333
EXP2_C0 = 127.0 * 128.0 - 64.0 - 0.5440619381193426
EXP2_C1 = 1.5 * 2**30

# SCALE_RECIP_ANT constants (Chebyshev seed over x*bitcast(~x) in [-4.5,-4])
SR_C0 = -0.23549792
SR_C1 = 2.0017324


def _exp2_reference(in0, in1, s0, s1, imm2):
    F = np.float32
    u = in0.astype(F)
    z = (u + F(s0)).astype(F)
    t = (z + F(s1)).astype(F)
    r = (t - F(s1)).astype(F)
    f = (z - r).astype(F)
    g = (f * f).astype(F)
    w = (g - in1.astype(F)).astype(F)
    h = (w * F(imm2)).astype(F)
    return (z + h).astype(F)


def _scale_recip_reference(in0, in1, s0, s1, imm2):
    F = np.float32
    x = in1.astype(F)
    nx = (~x.view(np.int32)).view(F)
    y0 = (nx * F(s0)).astype(F)
    y1 = (y0 * (F(s1) - (x * y0).astype(F)).astype(F)).astype(F)
    return (in0.astype(F) * y1).astype(F)


def _register_ops():
    import concourse.dve_ops as dve_ops
    from concourse.dve_spec import (
        Spec, Src0, Src1, C0, C1, C2, C3, AluOp, Bin, lower,
        _spill_c3_to_src1,
    )
    from concourse.dve_uop import DveOpSpec

    def add_op(name, spec):
        if name in dve_ops._SUB_OPCODE_FOR_NAME:
            for op in dve_ops.OPS:
                if op.name == name:
                    return op
            raise RuntimeError(f"{name}: row registered but op missing")
        row = dve_ops._CUSTOM_DVE_ROW_BASE + len(dve_ops.OPS)
        assert row < 0x20
        dve_ops._SUB_OPCODE_FOR_NAME[name] = row
        shas = {}
        for ver in ("v3", "v4"):
            s = DveOpSpec(name=name, opcode=row, uops=lower(spec, ver=ver),
                          rd1_en=True)
            shas[ver] = s.sha(ver)
        op = dve_ops.DveOp(name, spec, subdim=False, uops_sha=shas)
        dve_ops.OPS.append(op)
        dve_ops.CUSTOM_DVE_SPECS[name] = spec
        return op

    # out_bits(int16) = round(z + a*(f^2 - c3)), z = in + C0, f = frac via
    # double-magic; bit pattern read back as bf16 equals 2^(in/128).
    z = Src0 + C0
    t = z + C1
    r = t - C1
    f = z - r
    g = f * f
    w = g - C3
    h = w * C2
    exp2_spec = Spec(body=_spill_c3_to_src1(z + h), reference=_exp2_reference)

    # out = in0 * recip(in1): y0 = C0*bitcast(~x); y1 = y0*(C1 - x*y0)
    nx = Bin(AluOp.BITWISE_NOT, Src1, Src1)
    y0 = nx * C0
    y1 = y0 * (C1 - Src1 * y0)
    sr_spec = Spec(body=Src0 * y1, reference=_scale_recip_reference)

    return add_op("EXP2_BF16_ANT", exp2_spec), add_op("SCALE_RECIP_ANT", sr_spec)


EXP2_OP, SCALE_RECIP_OP = _register_ops()


def _chunks(total, step):
    res = []
    o = 0
    while o < total:
        res.append((o, min(step, total - o)))
        o += min(step, total - o)
    return res


def build():
    nc = bacc.Bacc()
    dp = nc.declare_dram_parameter
    xT = dp("xT", [C, T], BF16, isOutput=False)
    qAT = dp("qAT", [C, RANK], BF16, isOutput=False)
    kAT = dp("kAT", [C, RANK], BF16, isOutput=False)
    vAT = dp("vAT", [C, RANK], BF16, isOutput=False)
    qBT = dp("qBT", [RANK, G], BF16, isOutput=False)
    kBT = dp("kBT", [RANK, G], BF16, isOutput=False)
    vBT = dp("vBT", [RANK, G], BF16, isOutput=False)
    cAT = dp("cAT", [G, RANK], BF16, isOutput=False)
    cBT = dp("cBT", [RANK, C], BF16, isOutput=False)
    # causal masking is done ON THE PE: for each diagonal 128x128 block the
    # score matmul accumulates  negid^T @ masknc = -8000 * [k > q]  on top of
    # the scores, so both exp paths produce ~2^-62 for non-causal entries.
    negid = dp("negid", [P, P], BF16, isOutput=False)
    masknc = dp("masknc", [P, P], BF16, isOutput=False)
    out = dp("out", [T, C], BF16, isOutput=True)

    Exp = mybir.ActivationFunctionType.Exp

    with TileContext(nc) as tc:
        with tc.tile_pool(name="sb", bufs=1) as sb:
            vAT_sb0 = sb.tile([P, 8, RANK], BF16, tag="vAT")
            nc.gpsimd.dma_start(out=vAT_sb0, in_=vAT.rearrange("(co ci) r -> ci co r", ci=P))
            xT_sb = sb.tile([P, 8, T], BF16, tag="xT")
            xTr = xT.rearrange("(co ci) t -> ci co t", ci=P)
            for cc in range(8):
                nc.gpsimd.dma_start(out=xT_sb[:, cc, :], in_=xTr[:, cc, :])
            qAT_sb = sb.tile([P, 8, RANK], BF16, tag="qAT")
            nc.gpsimd.dma_start(out=qAT_sb, in_=qAT.rearrange("(co ci) r -> ci co r", ci=P))
            kAT_sb = sb.tile([P, 8, RANK], BF16, tag="kAT")
            nc.gpsimd.dma_start(out=kAT_sb, in_=kAT.rearrange("(co ci) r -> ci co r", ci=P))
            vAT_sb = vAT_sb0
            qBT_sb = sb.tile([RANK, G], BF16, tag="qBT")
            nc.gpsimd.dma_start(out=qBT_sb, in_=qBT[:, :])
            kBT_sb = sb.tile([RANK, G], BF16, tag="kBT")
            nc.gpsimd.dma_start(out=kBT_sb, in_=kBT[:, :])
            vBT_sb = sb.tile([RANK, G], BF16, tag="vBT")
            nc.gpsimd.dma_start(out=vBT_sb, in_=vBT[:, :])
            cAT2_sb = sb.tile([P, NPAIR, RANK], BF16, tag="cAT")
            nc.gpsimd.dma_start(out=cAT2_sb, in_=cAT.rearrange("(hp p) r -> p hp r", p=P))
            cBT_sb = sb.tile([RANK, C], BF16, tag="cBT")
            nc.gpsimd.dma_start(out=cBT_sb, in_=cBT[:, :])
            negid_sb = sb.tile([P, P], BF16, tag="negid")
            nc.gpsimd.dma_start(out=negid_sb, in_=negid[:, :])
            masknc_sb = sb.tile([P, P], BF16, tag="masknc")
            nc.gpsimd.dma_start(out=masknc_sb, in_=masknc[:, :])

            QT2 = sb.tile([P, NPAIR, T], BF16, tag="QT2")
            KT2 = sb.tile([P, NPAIR, T], BF16, tag="KT2")
            YT2 = sb.tile([P, NPAIR, T], BF16, tag="YT2")
            V_sb = sb.tile([P, 16, HPC, P], BF16, tag="Vsb")
            tTq = sb.tile([P, T], BF16, tag="tTq")
            tTk = sb.tile([P, T], BF16, tag="tTk")
            tTv = sb.tile([P, T], BF16, tag="tTv")
            rT_sb = sb.tile([P, T], BF16, tag="rT")

            nc.gpsimd.memset(V_sb[:, :, :, 64:P], 1.0)
            c3_sb = sb.tile([P, 1], F32, tag="c3")
            nc.gpsimd.memset(c3_sb, EXP2_C3)

            def exp2_dve(out_bf16_ap, in_psum_ap):
                nc.vector._custom_dve(
                    EXP2_OP, out=out_bf16_ap.bitcast(I16),
                    in0=in_psum_ap, in1=c3_sb[:, :],
                    s0=EXP2_C0, s1=EXP2_C1, imm2=EXP2_A,
                )

            def scale_recip_dve(out_ap, ytilde_psum_ap, den_sbuf_ap):
                nc.vector._custom_dve(
                    SCALE_RECIP_OP, out=out_ap,
                    in0=ytilde_psum_ap, in1=den_sbuf_ap,
                    s0=SR_C0, s1=SR_C1,
                )

            # ---- phase A: t^T = A @ x^T for q,k,v ----
            evac_tick = [0]

            def evac(out_ap, in_ap):
                # alternate PSUM evacuations between ScalarE and VectorE
                if evac_tick[0] % 2 == 0:
                    nc.scalar.copy(out=out_ap, in_=in_ap)
                else:
                    nc.vector.tensor_copy(out=out_ap, in_=in_ap)
                evac_tick[0] += 1

            with (
                tc.tile_pool(name="psA", bufs=2, space="PSUM") as psA,
                tc.tile_pool(name="psB", bufs=2, space="PSUM") as psB,
                tc.tile_pool(name="psV", bufs=2, space="PSUM") as psV,
            ):
                for pi, (AT_sb, tT) in enumerate(
                    ((vAT_sb, tTv), (qAT_sb, tTq), (kAT_sb, tTk))
                ):
                    for th in range(2):
                        h0 = th * 1024
                        pt = psA.tile([P, 1024], F32, tag="psA",
                                      name=f"psA{pi}_{th}")
                        for cc in range(8):
                            for t0, tw in _chunks(1024, 512):
                                nc.tensor.matmul(
                                    pt[:, t0:t0 + tw],
                                    AT_sb[:, cc, :],
                                    xT_sb[:, cc, h0 + t0:h0 + t0 + tw],
                                    start=(cc == 0), stop=(cc == 7),
                                )
                        evac(tT[:, h0:h0 + 512], pt[:, 0:512])
                        evac(tT[:, h0 + 512:h0 + 1024], pt[:, 512:1024])

                # ---- phase B: V keys-major with ones column ----
                for ti in range(16):
                    pv = psV.tile([P, G], F32, tag="psV")
                    nc.tensor.matmul(
                        pv, tTv[:, ti * 128:(ti + 1) * 128], vBT_sb,
                        start=True, stop=True,
                    )
                    # rank-3 strided dest -> keep on VectorE (ScalarE copy
                    # faults on this AP shape)
                    nc.vector.tensor_copy(
                        out=V_sb[:, ti, :, 0:64],
                        in_=pv.rearrange("p (h d) -> p h d", d=64),
                    )

                # ---- phase B: pair-stacked Q^T, K^T  (M=128 = 2 heads) ----
                for BT_sb, dest, tT in ((qBT_sb, QT2, tTq), (kBT_sb, KT2, tTk)):
                    for hp in range(NPAIR):
                        for t0, tw in _chunks(T, 512):
                            p2 = psB.tile([P, 512], F32, tag="psB")
                            nc.tensor.matmul(
                                p2[:, :tw],
                                BT_sb[:, hp * P:(hp + 1) * P],
                                tT[:, t0:t0 + tw],
                                start=True, stop=True,
                            )
                            evac(dest[:, hp, t0:t0 + tw], p2[:, :tw])

            # ---- attention: (head-pair, 512-query panel) tiles ----
            # For panel j the key strips are kb = 0..4j+3; strips are
            # processed in PAIRS: each psS tile [128, 2, 512] holds two
            # strips' scores (2 psum banks), giving a 2-pair (4-strip)
            # pipeline so the exp latency on ScalarE/VectorE never stalls
            # the PE, and one exp instruction covers both strips.  The
            # last 4 strips of each panel carry the causal diagonal; the
            # score matmul for those accumulates -8000*[k>q] via a second
            # small matmul (negid^T @ masknc) so no post-exp mask is
            # needed.  pvt rows 64:128 collect the softmax denominator
            # (V ones-rows); per (panel, head): ScalarE stages it to SBUF
            # and one fused DVE op writes Y^T = ytilde * recip(den).
            with (
                tc.tile_pool(name="psS", bufs=2, space="PSUM") as psS,
                tc.tile_pool(name="psPV", bufs=4, space="PSUM") as psPV,
                tc.tile_pool(name="ptp", bufs=4) as ptp,
                tc.tile_pool(name="den", bufs=4) as denp,
            ):
                for hp in range(NPAIR):
                    for j in range(4):
                        q0 = 512 * j
                        nkb = 4 * j + 4
                        pvt = [psPV.tile([P, 512], F32, tag="pv",
                                         name=f"pv{hp}_{j}_{e}")
                               for e in range(2)]

                        def strip_w(kb):
                            return q0 + 512 - max(q0, 128 * kb)

                        def emit_scores(kb, e, sps_slice):
                            qlo = max(q0, 128 * kb)
                            w = q0 + 512 - qlo
                            diag = kb >= 4 * j
                            nc.tensor.matmul(
                                sps_slice[:, 0:w],
                                KT2[64 * e:64 * e + 64, hp,
                                    kb * 128:(kb + 1) * 128],
                                QT2[64 * e:64 * e + 64, hp, qlo:qlo + w],
                                start=True, stop=not diag,
                            )
                            if diag:
                                nc.tensor.matmul(
                                    sps_slice[:, 0:P], negid_sb, masknc_sb,
                                    start=False, stop=True,
                                )

                        def emit_exp(pi, e, sps, ptile):
                            # one activation covers both strips of the pair
                            # (flat view across the two 512-col slices)
                            kb0, kb1 = 2 * pi, 2 * pi + 1
                            w1 = strip_w(kb1)
                            wflat = 512 + w1
                            flat_s = sps.rearrange("p s n -> p (s n)")
                            flat_p = ptile.rearrange("p s n -> p (s n)")
                            # ~10% of the DVE lane's pairs go to ScalarE to
                            # balance the two engines
                            on_scalar = (e == 0) or (pi % 8 == 5)
                            if on_scalar:
                                nc.scalar.activation(
                                    flat_p[:, :wflat], flat_s[:, :wflat], Exp,
                                    scale=LN2_OVER_128)
                            else:
                                exp2_dve(flat_p[:, :wflat], flat_s[:, :wflat])

                        def emit_pv(kb, e, pt_slice):
                            qlo = max(q0, 128 * kb)
                            w = q0 + 512 - qlo
                            c0 = qlo - q0
                            nc.tensor.matmul(
                                pvt[e][:, c0:c0 + w],
                                V_sb[:, kb, 2 * hp + e, :],
                                pt_slice[:, 0:w],
                                start=(kb == 0), stop=(kb == nkb - 1),
                            )

                        npair_kb = nkb // 2
                        hist = {}
                        for pi in range(npair_kb):
                            sps = [psS.tile([P, 2, 512], F32, tag="s",
                                            name=f"s{hp}_{j}_{pi}_{e}")
                                   for e in range(2)]
                            ptile = [ptp.tile([P, 2, 512], BF16, tag="pt",
                                              name=f"p{hp}_{j}_{pi}_{e}")
                                    for e in range(2)]
                            for s in range(2):
                                emit_scores(2 * pi + s, 0, sps[0][:, s, :])
                                emit_scores(2 * pi + s, 1, sps[1][:, s, :])
                            emit_exp(pi, 0, sps[0], ptile[0])
                            emit_exp(pi, 1, sps[1], ptile[1])
                            hist[pi] = ptile
                            if pi >= 1:
                                for s in range(2):
                                    emit_pv(2 * (pi - 1) + s, 0,
                                            hist[pi - 1][0][:, s, :])
                                    emit_pv(2 * (pi - 1) + s, 1,
                                            hist[pi - 1][1][:, s, :])
                        pi = npair_kb - 1
                        for s in range(2):
                            emit_pv(2 * pi + s, 0, hist[pi][0][:, s, :])
                            emit_pv(2 * pi + s, 1, hist[pi][1][:, s, :])
                        for e in range(2):
                            den = denp.tile([64, 512], F32, tag="den",
                                            name=f"dn{hp}_{j}_{e}")
                            nc.scalar.copy(out=den, in_=pvt[e][64:P, :])
                            if e == 1:
                                # custom-DVE out cannot partition-shift:
                                # write rows 0:64 scratch, ScalarE shifts up
                                ysc = denp.tile([64, 512], BF16, tag="ysc",
                                                name=f"ys{hp}_{j}")
                                scale_recip_dve(
                                    ysc, pvt[e][0:64, :], den)
                                nc.scalar.copy(
                                    out=YT2[64:P, hp, q0:q0 + 512], in_=ysc)
                                continue
                            scale_recip_dve(
                                YT2[64 * e:64 * e + 64, hp, q0:q0 + 512],
                                pvt[e][0:64, :],
                                den,
                            )

            # ---- phase D: c_proj ----
            with tc.tile_pool(name="psD", bufs=1, space="PSUM") as psD:
                pr = psD.tile([P, T], F32, tag="r")
                for hp in range(NPAIR):
                    for t0, tw in _chunks(T, 512):
                        nc.tensor.matmul(
                            pr[:, t0:t0 + tw], cAT2_sb[:, hp, :],
                            YT2[:, hp, t0:t0 + tw],
                            start=(hp == 0), stop=(hp == NPAIR - 1),
                        )
                for t0, tw in _chunks(T, 512):
                    evac(rT_sb[:, t0:t0 + tw], pr[:, t0:t0 + tw])
            with (
                tc.tile_pool(name="psO", bufs=6, space="PSUM") as psO,
                tc.tile_pool(name="ost", bufs=6) as ost,
            ):
                for ti in range(16):
                    for nn in range(2):
                        po = psO.tile([P, 512], F32, tag="o")
                        nc.tensor.matmul(
                            po, rT_sb[:, ti * 128:(ti + 1) * 128],
                            cBT_sb[:, nn * 512:(nn + 1) * 512],
                            start=True, stop=True,
                        )
                        ob = ost.tile([P, 512], BF16, tag="ob")
                        evac(ob, po)
                        nc.sync.dma_start(
                            out=out[ti * 128:(ti + 1) * 128,
                                    nn * 512:(nn + 1) * 512],
                            in_=ob,
                        )
    nc.finalize()
    return nc


def make_in_maps(x, qA, qB, kA, kB, vA, vB, cA, cB):
    x, qA, qB, kA, kB, vA, vB, cA, cB = [
        np.asarray(a, dtype=np.float32) for a in (x, qA, qB, kA, kB, vA, vB, cA, cB)
    ]
    negid = (-8000.0 * np.eye(P, dtype=np.float32)).astype(BF)
    masknc = np.where(
        np.arange(P)[:, None] > np.arange(P)[None, :], 1.0, 0.0
    ).astype(BF)
    qATn = np.ascontiguousarray(qA.T).astype(BF)
    kATn = np.ascontiguousarray(kA.T).astype(BF)
    vATn = np.ascontiguousarray(vA.T).astype(BF)
    cBTn = np.ascontiguousarray(cB.T).astype(BF)
    in_maps = []
    for c in range(NCORES):
        b, g = divmod(c, 2)
        sl = slice(g * G, (g + 1) * G)
        in_maps.append({
            "xT": np.ascontiguousarray(x[b].T).astype(BF),
            "qAT": qATn, "kAT": kATn, "vAT": vATn,
            "qBT": (np.ascontiguousarray(qB[sl, :].T) * KAPPA).astype(BF),
            "kBT": np.ascontiguousarray(kB[sl, :].T).astype(BF),
            "vBT": np.ascontiguousarray(vB[sl, :].T).astype(BF),
            "cAT": np.ascontiguousarray(cA[:, sl].T).astype(BF),
            "cBT": cBTn,
            "negid": negid,
            "masknc": masknc,
        })
    return in_maps


def combine(parts):
    return np.stack(
        [parts[2 * b].astype(np.float32) + parts[2 * b + 1].astype(np.float32)
         for b in range(B)], axis=0)


def kernel(x, qA, qB, kA, kB, vA, vB, cA, cB):
    global _NC_CACHE
    if _NC_CACHE is None:
        _NC_CACHE = build()
    in_maps = make_in_maps(x, qA, qB, kA, kB, vA, vB, cA, cB)
    res = run_bass_kernel_spmd(_NC_CACHE, in_maps, list(range(NCORES))).results
    return combine([res[c]["out"] for c in range(NCORES)])


# revision 11
# speedup vs baseline: 1.4182x; 1.4182x over previous
"""HarmonicCausalSelfAttention on 8 TRN2 NeuronCores.

Sharding: core c -> (batch b = c//2, head-group g = c%2); each core computes
attention for 8 heads of one batch and a full-width partial of the output
projection; the host sums the two partials per batch (the rank-128 c-proj
intermediate is linear, so out = (r_g0 + r_g1) @ cB^T = part_g0 + part_g1).

Layout: everything transposed so no on-device transposes are needed.
  stage1:  t^T[rank, T] = A @ x^T       (x^T prepared host-side, bf16)
  stage2:  QT2/KT2[128 = 2 heads x 64d, hp, T] pair-stacked; V keys-major
           [keys, 128] where cols 64:128 are ALL ONES (so PV psum rows
           64:128 hold the softmax denominator replicated 64x).
  attn per head pair, per q-half (hc), per 128-key strip kb:
           S^T[keys, q] = K_kb @ Q^T   (half-array K=64; the e=0 and e=1
           head lanes use partition rows 0:64 / 64:128, emitted
           back-to-back so the PE runs them concurrently as row-group
           tiles)
           softmax exp is SPLIT ACROSS ENGINES: the e=0 lane runs native
           exp on ScalarE (PSUM->SBUF bf16); the e=1 lane runs a CUSTOM
           DVE op (EXP2_BF16_ANT, registered at import) that constructs
           the bf16 BIT PATTERN of 2^y arithmetically -- magic-constant
           floor/frac split + quadratic mantissa correction, written
           through an f32->int16 convert and bitcast back to bf16
           (max rel err 5.4e-3, rms 2.2e-3; scores are pre-scaled by
           kappa = 16/ln2 host-side via the Q projection so psum already
           holds y*128 = log2(e^{S/8})*128). Causal diag masking is
           POST-exp by a bf16 0/1 multiply on GpSimd.
           PV: psum[128, 512q] += V_kb^T @ P^T_kb; normalization: ScalarE
           stages the replicated denominator rows 64:128 -> SBUF, then one
           custom DVE op (SCALE_RECIP_ANT: bitwise-NOT reciprocal seed +
           1 Newton step + multiply, 6 stages, max rel err 1.7e-3)
           computes Y^T = ytilde * (1/den) straight out of PSUM.
  c_proj:  r^T[rank, T] = sum_hp cA_hp2[128d x rank] @ YT2_hp; out chunks
           = r^T.T @ cB^T, cast bf16 (ScalarE/DVE alternating), DMA out.
PSUM-evacuation copies everywhere are alternated between ScalarE and
VectorE so neither engine paces the loop; TensorE is the critical path.
Output returned in bf16; host sums partials in f32.

Measured on 8-core trn2 (neuron-profile): 608 us (session-start)
-> 338 us (prev session) -> this version targets ~150 us.
"""

import math

import numpy as np
import ml_dtypes

import concourse.bass as bass
from concourse import bacc
import concourse.mybir as mybir
from concourse.tile import TileContext
from concourse.bass_utils import run_bass_kernel_spmd

B, T, C = 4, 2048, 1024
NH, HD = 16, 64
RANK = 128
NCORES = 8
HPC = 8          # heads per core
NPAIR = 4        # head pairs per core
G = 512          # C columns per head group
P = 128
F32 = mybir.dt.float32
BF16 = mybir.dt.bfloat16
I16 = mybir.dt.int16
BF = ml_dtypes.bfloat16

# scores arrive in psum pre-scaled so that psum = y*128, y = log2(exp(S/8))
KAPPA = 16.0 / math.log(2.0)          # folded into qB host-side
LN2_OVER_128 = math.log(2.0) / 128.0  # ScalarE exp scale for the same psum

_NC_CACHE = None

# ---------------------------------------------------------------------------
# custom DVE ops, registered into concourse.dve_ops at import time
# ---------------------------------------------------------------------------

# EXP2_BF16_ANT constants (fit numerically; see docstring)
EXP2_A = 0.002689572895776347
EXP2_C3 = -19931.876633283333
EXP2_C0 = 127.0 * 128.0 - 64.0 - 0.5440619381193426
EXP2_C1 = 1.5 * 2**30

# SCALE_RECIP_ANT constants (Chebyshev seed over x*bitcast(~x) in [-4.5,-4])
SR_C0 = -0.23549792
SR_C1 = 2.0017324


def _exp2_reference(in0, in1, s0, s1, imm2):
    F = np.float32
    u = in0.astype(F)
    z = (u + F(s0)).astype(F)
    t = (z + F(s1)).astype(F)
    r = (t - F(s1)).astype(F)
    f = (z - r).astype(F)
    g = (f * f).astype(F)
    w = (g - in1.astype(F)).astype(F)
    h = (w * F(imm2)).astype(F)
    return (z + h).astype(F)


def _scale_recip_reference(in0, in1, s0, s1, imm2):
    F = np.float32
    x = in1.astype(F)
    nx = (~x.view(np.int32)).view(F)
    y0 = (nx * F(s0)).astype(F)
    y1 = (y0 * (F(s1) - (x * y0).astype(F)).astype(F)).astype(F)
    return (in0.astype(F) * y1).astype(F)


def _register_ops():
    import concourse.dve_ops as dve_ops
    from concourse.dve_spec import (
        Spec, Src0, Src1, C0, C1, C2, C3, AluOp, Bin, lower,
        _spill_c3_to_src1,
    )
    from concourse.dve_uop import DveOpSpec

    def add_op(name, spec):
        if name in dve_ops._SUB_OPCODE_FOR_NAME:
            for op in dve_ops.OPS:
                if op.name == name:
                    return op
            raise RuntimeError(f"{name}: row registered but op missing")
        row = dve_ops._CUSTOM_DVE_ROW_BASE + len(dve_ops.OPS)
        assert row < 0x20
        dve_ops._SUB_OPCODE_FOR_NAME[name] = row
        shas = {}
        for ver in ("v3", "v4"):
            s = DveOpSpec(name=name, opcode=row, uops=lower(spec, ver=ver),
                          rd1_en=True)
            shas[ver] = s.sha(ver)
        op = dve_ops.DveOp(name, spec, subdim=False, uops_sha=shas)
        dve_ops.OPS.append(op)
        dve_ops.CUSTOM_DVE_SPECS[name] = spec
        return op

    # out_bits(int16) = round(z + a*(f^2 - c3)), z = in + C0, f = frac via
    # double-magic; bit pattern read back as bf16 equals 2^(in/128).
    z = Src0 + C0
    t = z + C1
    r = t - C1
    f = z - r
    g = f * f
    w = g - C3
    h = w * C2
    exp2_spec = Spec(body=_spill_c3_to_src1(z + h), reference=_exp2_reference)

    # out = in0 * recip(in1): y0 = C0*bitcast(~x); y1 = y0*(C1 - x*y0)
    nx = Bin(AluOp.BITWISE_NOT, Src1, Src1)
    y0 = nx * C0
    y1 = y0 * (C1 - Src1 * y0)
    sr_spec = Spec(body=Src0 * y1, reference=_scale_recip_reference)

    return add_op("EXP2_BF16_ANT", exp2_spec), add_op("SCALE_RECIP_ANT", sr_spec)


EXP2_OP, SCALE_RECIP_OP = _register_ops()


def _chunks(total, step):
    res = []
    o = 0
    while o < total:
        res.append((o, min(step, total - o)))
        o += min(step, total - o)
    return res


def build():
    nc = bacc.Bacc()
    dp = nc.declare_dram_parameter
    xT = dp("xT", [C, T], BF16, isOutput=False)
    qAT = dp("qAT", [C, RANK], BF16, isOutput=False)
    kAT = dp("kAT", [C, RANK], BF16, isOutput=False)
    vAT = dp("vAT", [C, RANK], BF16, isOutput=False)
    qBT = dp("qBT", [RANK, G], BF16, isOutput=False)
    kBT = dp("kBT", [RANK, G], BF16, isOutput=False)
    vBT = dp("vBT", [RANK, G], BF16, isOutput=False)
    cAT = dp("cAT", [G, RANK], BF16, isOutput=False)
    cBT = dp("cBT", [RANK, C], BF16, isOutput=False)
    # causal masking is done ON THE PE: for each diagonal 128x128 block the
    # score matmul accumulates  negid^T @ masknc = -8000 * [k > q]  on top of
    # the scores, so both exp paths produce ~2^-62 for non-causal entries.
    negid = dp("negid", [P, P], BF16, isOutput=False)
    masknc = dp("masknc", [P, P], BF16, isOutput=False)
    out = dp("out", [T, C], BF16, isOutput=True)

    Exp = mybir.ActivationFunctionType.Exp

    with TileContext(nc) as tc:
        with tc.tile_pool(name="sb", bufs=1) as sb:
            vAT_sb0 = sb.tile([P, 8, RANK], BF16, tag="vAT")
            nc.gpsimd.dma_start(out=vAT_sb0, in_=vAT.rearrange("(co ci) r -> ci co r", ci=P))
            xT_sb = sb.tile([P, 8, T], BF16, tag="xT")
            xTr = xT.rearrange("(co ci) t -> ci co t", ci=P)
            for cc in range(8):
                nc.gpsimd.dma_start(out=xT_sb[:, cc, :], in_=xTr[:, cc, :])
            qAT_sb = sb.tile([P, 8, RANK], BF16, tag="qAT")
            nc.gpsimd.dma_start(out=qAT_sb, in_=qAT.rearrange("(co ci) r -> ci co r", ci=P))
            kAT_sb = sb.tile([P, 8, RANK], BF16, tag="kAT")
            nc.gpsimd.dma_start(out=kAT_sb, in_=kAT.rearrange("(co ci) r -> ci co r", ci=P))
            vAT_sb = vAT_sb0
            qBT_sb = sb.tile([RANK, G], BF16, tag="qBT")
            nc.gpsimd.dma_start(out=qBT_sb, in_=qBT[:, :])
            kBT_sb = sb.tile([RANK, G], BF16, tag="kBT")
            nc.gpsimd.dma_start(out=kBT_sb, in_=kBT[:, :])
            vBT_sb = sb.tile([RANK, G], BF16, tag="vBT")
            nc.gpsimd.dma_start(out=vBT_sb, in_=vBT[:, :])
            cAT2_sb = sb.tile([P, NPAIR, RANK], BF16, tag="cAT")
            nc.gpsimd.dma_start(out=cAT2_sb, in_=cAT.rearrange("(hp p) r -> p hp r", p=P))
            cBT_sb = sb.tile([RANK, C], BF16, tag="cBT")
            nc.gpsimd.dma_start(out=cBT_sb, in_=cBT[:, :])
            negid_sb = sb.tile([P, P], BF16, tag="negid")
            nc.gpsimd.dma_start(out=negid_sb, in_=negid[:, :])
            masknc_sb = sb.tile([P, P], BF16, tag="masknc")
            nc.gpsimd.dma_start(out=masknc_sb, in_=masknc[:, :])

            QT2 = sb.tile([P, NPAIR, T], BF16, tag="QT2")
            KT2 = sb.tile([P, NPAIR, T], BF16, tag="KT2")
            YT2 = sb.tile([P, NPAIR, T], BF16, tag="YT2")
            V_sb = sb.tile([P, 16, HPC, P], BF16, tag="Vsb")
            tTq = sb.tile([P, T], BF16, tag="tTq")
            tTk = sb.tile([P, T], BF16, tag="tTk")
            tTv = sb.tile([P, T], BF16, tag="tTv")
            rT_sb = sb.tile([P, T], BF16, tag="rT")

            nc.gpsimd.memset(V_sb[:, :, :, 64:P], 1.0)
            c3_sb = sb.tile([P, 1], F32, tag="c3")
            nc.gpsimd.memset(c3_sb, EXP2_C3)

            def exp2_dve(out_bf16_ap, in_psum_ap):
                nc.vector._custom_dve(
                    EXP2_OP, out=out_bf16_ap.bitcast(I16),
                    in0=in_psum_ap, in1=c3_sb[:, :],
                    s0=EXP2_C0, s1=EXP2_C1, imm2=EXP2_A,
                )

            def scale_recip_dve(out_ap, ytilde_psum_ap, den_sbuf_ap):
                nc.vector._custom_dve(
                    SCALE_RECIP_OP, out=out_ap,
                    in0=ytilde_psum_ap, in1=den_sbuf_ap,
                    s0=SR_C0, s1=SR_C1,
                )

            # ---- phase A: t^T = A @ x^T for q,k,v ----
            evac_tick = [0]

            def evac(out_ap, in_ap):
                # alternate PSUM evacuations between ScalarE and VectorE
                if evac_tick[0] % 2 == 0:
                    nc.scalar.copy(out=out_ap, in_=in_ap)
                else:
                    nc.vector.tensor_copy(out=out_ap, in_=in_ap)
                evac_tick[0] += 1

            with (
                tc.tile_pool(name="psA", bufs=2, space="PSUM") as psA,
                tc.tile_pool(name="psB", bufs=2, space="PSUM") as psB,
                tc.tile_pool(name="psV", bufs=2, space="PSUM") as psV,
            ):
                for pi, (AT_sb, tT) in enumerate(
                    ((vAT_sb, tTv), (qAT_sb, tTq), (kAT_sb, tTk))
                ):
                    for th in range(2):
                        h0 = th * 1024
                        pt = psA.tile([P, 1024], F32, tag="psA",
                                      name=f"psA{pi}_{th}")
                        for cc in range(8):
                            for t0, tw in _chunks(1024, 512):
                                nc.tensor.matmul(
                                    pt[:, t0:t0 + tw],
                                    AT_sb[:, cc, :],
                                    xT_sb[:, cc, h0 + t0:h0 + t0 + tw],
                                    start=(cc == 0), stop=(cc == 7),
                                )
                        evac(tT[:, h0:h0 + 512], pt[:, 0:512])
                        evac(tT[:, h0 + 512:h0 + 1024], pt[:, 512:1024])

                # ---- phase B: V keys-major with ones column ----
                for ti in range(16):
                    pv = psV.tile([P, G], F32, tag="psV")
                    nc.tensor.matmul(
                        pv, tTv[:, ti * 128:(ti + 1) * 128], vBT_sb,
                        start=True, stop=True,
                    )
                    # rank-3 strided dest -> keep on VectorE (ScalarE copy
                    # faults on this AP shape)
                    nc.vector.tensor_copy(
                        out=V_sb[:, ti, :, 0:64],
                        in_=pv.rearrange("p (h d) -> p h d", d=64),
                    )

                # ---- phase B: pair-stacked Q^T, K^T  (M=128 = 2 heads) ----
                for BT_sb, dest, tT in ((qBT_sb, QT2, tTq), (kBT_sb, KT2, tTk)):
                    for hp in range(NPAIR):
                        for t0, tw in _chunks(T, 512):
                            p2 = psB.tile([P, 512], F32, tag="psB")
                            nc.tensor.matmul(
                                p2[:, :tw],
                                BT_sb[:, hp * P:(hp + 1) * P],
                                tT[:, t0:t0 + tw],
                                start=True, stop=True,
                            )
                            evac(dest[:, hp, t0:t0 + tw], p2[:, :tw])

            # ---- attention: (head-pair, 512-query panel) tiles ----
            # For panel j the key strips are kb = 0..4j+3; strips are
            # processed in PAIRS: each psS tile [128, 2, 512] holds two
            # strips' scores (2 psum banks), giving a 2-pair (4-strip)
            # pipeline so the exp latency on ScalarE/VectorE never stalls
            # the PE, and one exp instruction covers both strips.  The
            # last 4 strips of each panel carry the causal diagonal; the
            # score matmul for those accumulates -8000*[k>q] via a second
            # small matmul (negid^T @ masknc) so no post-exp mask is
            # needed.  pvt rows 64:128 collect the softmax denominator
            # (V ones-rows); per (panel, head): ScalarE stages it to SBUF
            # and one fused DVE op writes Y^T = ytilde * recip(den).
            with (
                tc.tile_pool(name="psS", bufs=3, space="PSUM") as psS,
                tc.tile_pool(name="psPV", bufs=2, space="PSUM") as psPV,
                tc.tile_pool(name="ptp", bufs=6) as ptp,
                tc.tile_pool(name="den", bufs=4) as denp,
            ):
                for hp in range(NPAIR):
                    for j in range(4):
                        q0 = 512 * j
                        nkb = 4 * j + 4
                        pvt = [psPV.tile([P, 512], F32, tag="pv",
                                         name=f"pv{hp}_{j}_{e}")
                               for e in range(2)]

                        def strip_w(kb):
                            return q0 + 512 - max(q0, 128 * kb)

                        def emit_scores(kb, e, sps_slice):
                            qlo = max(q0, 128 * kb)
                            w = q0 + 512 - qlo
                            diag = kb >= 4 * j
                            nc.tensor.matmul(
                                sps_slice[:, 0:w],
                                KT2[64 * e:64 * e + 64, hp,
                                    kb * 128:(kb + 1) * 128],
                                QT2[64 * e:64 * e + 64, hp, qlo:qlo + w],
                                start=True, stop=not diag,
                            )
                            if diag:
                                nc.tensor.matmul(
                                    sps_slice[:, 0:P], negid_sb, masknc_sb,
                                    start=False, stop=True,
                                )

                        def emit_exp(pi, e, sps, ptile):
                            # one activation covers both strips of the pair
                            # (flat view across the two 512-col slices)
                            kb0, kb1 = 2 * pi, 2 * pi + 1
                            w1 = strip_w(kb1)
                            wflat = 512 + w1
                            flat_s = sps.rearrange("p s n -> p (s n)")
                            flat_p = ptile.rearrange("p s n -> p (s n)")
                            # ~10% of the DVE lane's pairs go to ScalarE to
                            # balance the two engines
                            on_scalar = (e == 0) or (pi % 8 == 5)
                            if on_scalar:
                                nc.scalar.activation(
                                    flat_p[:, :wflat], flat_s[:, :wflat], Exp,
                                    scale=LN2_OVER_128)
                            else:
                                exp2_dve(flat_p[:, :wflat], flat_s[:, :wflat])

                        def emit_pv(kb, e, pt_slice):
                            qlo = max(q0, 128 * kb)
                            w = q0 + 512 - qlo
                            c0 = qlo - q0
                            nc.tensor.matmul(
                                pvt[e][:, c0:c0 + w],
                                V_sb[:, kb, 2 * hp + e, :],
                                pt_slice[:, 0:w],
                                start=(kb == 0), stop=(kb == nkb - 1),
                            )

                        npair_kb = nkb // 2
                        hist = {}
                        for pi in range(npair_kb):
                            sps = [psS.tile([P, 2, 512], F32, tag="s",
                                            name=f"s{hp}_{j}_{pi}_{e}")
                                   for e in range(2)]
                            ptile = [ptp.tile([P, 2, 512], BF16, tag="pt",
                                              name=f"p{hp}_{j}_{pi}_{e}")
                                    for e in range(2)]
                            for s in range(2):
                                emit_scores(2 * pi + s, 0, sps[0][:, s, :])
                                emit_scores(2 * pi + s, 1, sps[1][:, s, :])
                            emit_exp(pi, 0, sps[0], ptile[0])
                            emit_exp(pi, 1, sps[1], ptile[1])
                            hist[pi] = ptile
                            if pi >= 1:
                                for s in range(2):
                                    emit_pv(2 * (pi - 1) + s, 0,
                                            hist[pi - 1][0][:, s, :])
                                    emit_pv(2 * (pi - 1) + s, 1,
                                            hist[pi - 1][1][:, s, :])
                        pi = npair_kb - 1
                        for s in range(2):
                            emit_pv(2 * pi + s, 0, hist[pi][0][:, s, :])
                            emit_pv(2 * pi + s, 1, hist[pi][1][:, s, :])
                        for e in range(2):
                            den = denp.tile([64, 512], F32, tag="den",
                                            name=f"dn{hp}_{j}_{e}")
                            nc.scalar.copy(out=den, in_=pvt[e][64:P, :])
                            if e == 1:
                                # custom-DVE out cannot partition-shift:
                                # write rows 0:64 scratch, ScalarE shifts up
                                ysc = denp.tile([64, 512], BF16, tag="ysc",
                                                name=f"ys{hp}_{j}")
                                scale_recip_dve(
                                    ysc, pvt[e][0:64, :], den)
                                nc.scalar.copy(
                                    out=YT2[64:P, hp, q0:q0 + 512], in_=ysc)
                                continue
                            scale_recip_dve(
                                YT2[64 * e:64 * e + 64, hp, q0:q0 + 512],
                                pvt[e][0:64, :],
                                den,
                            )

            # ---- phase D: c_proj ----
            with tc.tile_pool(name="psD", bufs=1, space="PSUM") as psD:
                pr = psD.tile([P, T], F32, tag="r")
                for hp in range(NPAIR):
                    for t0, tw in _chunks(T, 512):
                        nc.tensor.matmul(
                            pr[:, t0:t0 + tw], cAT2_sb[:, hp, :],
                            YT2[:, hp, t0:t0 + tw],
                            start=(hp == 0), stop=(hp == NPAIR - 1),
                        )
                for t0, tw in _chunks(T, 512):
                    evac(rT_sb[:, t0:t0 + tw], pr[:, t0:t0 + tw])
            with (
                tc.tile_pool(name="psO", bufs=6, space="PSUM") as psO,
                tc.tile_pool(name="ost", bufs=6) as ost,
            ):
                for ti in range(16):
                    for nn in range(2):
                        po = psO.tile([P, 512], F32, tag="o")
                        nc.tensor.matmul(
                            po, rT_sb[:, ti * 128:(ti + 1) * 128],
                            cBT_sb[:, nn * 512:(nn + 1) * 512],
                            start=True, stop=True,
                        )
                        ob = ost.tile([P, 512], BF16, tag="ob")
                        evac(ob, po)
                        nc.sync.dma_start(
                            out=out[ti * 128:(ti + 1) * 128,
                                    nn * 512:(nn + 1) * 512],
                            in_=ob,
                        )
    nc.finalize()
    return nc


def make_in_maps(x, qA, qB, kA, kB, vA, vB, cA, cB):
    x, qA, qB, kA, kB, vA, vB, cA, cB = [
        np.asarray(a, dtype=np.float32) for a in (x, qA, qB, kA, kB, vA, vB, cA, cB)
    ]
    negid = (-8000.0 * np.eye(P, dtype=np.float32)).astype(BF)
    masknc = np.where(
        np.arange(P)[:, None] > np.arange(P)[None, :], 1.0, 0.0
    ).astype(BF)
    qATn = np.ascontiguousarray(qA.T).astype(BF)
    kATn = np.ascontiguousarray(kA.T).astype(BF)
    vATn = np.ascontiguousarray(vA.T).astype(BF)
    cBTn = np.ascontiguousarray(cB.T).astype(BF)
    in_maps = []
    for c in range(NCORES):
        b, g = divmod(c, 2)
        sl = slice(g * G, (g + 1) * G)
        in_maps.append({
            "xT": np.ascontiguousarray(x[b].T).astype(BF),
            "qAT": qATn, "kAT": kATn, "vAT": vATn,
            "qBT": (np.ascontiguousarray(qB[sl, :].T) * KAPPA).astype(BF),
            "kBT": np.ascontiguousarray(kB[sl, :].T).astype(BF),
            "vBT": np.ascontiguousarray(vB[sl, :].T).astype(BF),
            "cAT": np.ascontiguousarray(cA[:, sl].T).astype(BF),
            "cBT": cBTn,
            "negid": negid,
            "masknc": masknc,
        })
    return in_maps


def combine(parts):
    return np.stack(
        [parts[2 * b].astype(np.float32) + parts[2 * b + 1].astype(np.float32)
         for b in range(B)], axis=0)


def kernel(x, qA, qB, kA, kB, vA, vB, cA, cB):
    global _NC_CACHE
    if _NC_CACHE is None:
        _NC_CACHE = build()
    in_maps = make_in_maps(x, qA, qB, kA, kB, vA, vB, cA, cB)
    res = run_bass_kernel_spmd(_NC_CACHE, in_maps, list(range(NCORES))).results
    return combine([res[c]["out"] for c in range(NCORES)])
